# revision 2
# baseline (speedup 1.0000x reference)
"""Trainium2 Bass kernel for nn_CNN_Att_16887811408068 (v3).

Data-parallel over batch on 8 NeuronCores. One replicated f16 pair
table; all embedding fetches ride one fused SWDGE gather stream (the
Q7 descriptor-generation rate is the wall, ~9ns/idx). Tail chunk
first: its Asum min/max AllGather is issued after the second gather
so the collective completes under the CNN chunks. Asymmetric chunk
sizes keep the post-wall tail short. f16 compute throughout.
"""
import numpy as np

import concourse.bacc as bacc
import concourse.mybir as mybir
import concourse.tile as tile
from concourse.ap import AP
from concourse.bass_utils import run_bass_kernel_spmd

F32 = mybir.dt.float32
F16 = mybir.dt.float16
I16 = mybir.dt.int16
U8 = mybir.dt.uint8
AF = mybir.ActivationFunctionType
ALU = mybir.AluOpType
AX = mybir.AxisListType

# problem constants
V, D, WIN, P, CR = 50000, 100, 5, 411, 0.8
LOCAL, NF, GOUT, NCLS = 100, 100, 100, 2987
B, LL, LG = 256, P + WIN - 1, 411
NCORE = 8
BSH = B // NCORE                     # 32 batch rows per core

# local tail: positions p in [P0, P); token range t in [P0, LL)
NP_TAIL = 4                          # 411^-4 ~ 3.5e-11: ample for f16 tol
P0 = P - NP_TAIL                     # 407
TBLK = LL - P0                       # 8 token blocks (t values)
NTAIL = TBLK * BSH                   # 256 tail token slots
JOFF = 2 * BSH                       # col offset of t'=P0+2 block (judge range)
NJ = NP_TAIL * BSH                   # 128 judge cols

# gather stream: chunk 0 = tail, then CNN chunks of ROWS[i] batch rows
VPAIR = (V + 1) // 2                 # 25000 pair rows
EPAIR = 256                          # f16 elems per pair row (512B)
ROWS = [4, 6, 6, 6, 6, 2, 2]         # small first (PE warmup), small last
ROFF = np.cumsum([0] + ROWS).tolist()
NCH = [NTAIL] + [(r * LG + 127) // 128 * 128 for r in ROWS]
NIDX = sum(NCH)                      # 13824
IOFF = np.cumsum([0] + NCH).tolist()

N_TILES_OUT = [(i * 512, min(512, NCLS - i * 512)) for i in range((NCLS + 511) // 512)]

_CACHE = {}


def _wrap_idx(vals):
    n = len(vals)
    g = np.zeros((16, n // 16), np.int16)
    g[np.arange(n) % 16, np.arange(n) // 16] = vals.astype(np.int16)
    return np.tile(g, (8, 1))


def _win_ap(t, col0):
    """overlapping window view [1, NP_TAIL, WIN] starting at free col col0."""
    base = t[0:1, col0:col0 + NP_TAIL]
    return AP(base.tensor, base.offset, [list(base.ap[0]), [1, NP_TAIL], [1, WIN]])


def _build():
    nc = bacc.Bacc("TRN2", target_bir_lowering=False, debug=False,
                   num_devices=NCORE, num_swdge_queues=2)
    dt = nc.dram_tensor
    pairs16 = dt("pairs16", [VPAIR, EPAIR], F16, kind="ExternalInput")
    g_idx = dt("g_idx", [128, NIDX // 16], I16, kind="ExternalInput")
    g_mask = dt("g_mask", [128, NIDX // 128, 128], U8, kind="ExternalInput")
    id16_in = dt("id16", [128, 128], F16, kind="ExternalInput")
    wgt_in = dt("wgt", [1, NJ], F32, kind="ExternalInput")
    big16_in = dt("big16", [128, 1733], F16, kind="ExternalInput")
    big32_in = dt("big32", [128, 106], F32, kind="ExternalInput")
    f2_pack = dt("f2_pack", [401, NCLS], F16, kind="ExternalInput")
    y = dt("y", [BSH, NCLS], F32, kind="ExternalOutput")

    with tile.TileContext(nc) as tc:
        with tc.tile_pool(name="const", bufs=1) as cp, \
                tc.tile_pool(name="psC", bufs=1, space="PSUM") as psC, \
                tc.tile_pool(name="psB", bufs=1, space="PSUM") as psB, \
                tc.tile_pool(name="psM", bufs=1, space="PSUM") as psM, \
                tc.tile_pool(name="dram", bufs=2, space="DRAM") as dp:

            # ---- idx load, then first gathers, then remaining constants ----
            gi = cp.tile([128, NIDX // 16], I16)
            nc.sync.dma_start(gi[:], g_idx[:, :])

            G = []
            def gather(c):
                g = cp.tile([128, NCH[c] // 128, EPAIR], F16,
                            tag=f"G{c}", name=f"G{c}")
                nc.gpsimd.dma_gather(
                    out_ap=g[:], in_ap=pairs16[:, :],
                    idxs_ap=gi[:, IOFF[c] // 16:IOFF[c + 1] // 16],
                    num_idxs=NCH[c], num_idxs_reg=NCH[c], elem_size=EPAIR,
                    single_packet=False, queue_num=c % 2)
                G.append(g)
            gather(0)
            gather(1)

            gmk = cp.tile([128, NIDX // 128, 128], U8)
            nc.sync.dma_start(gmk[:], g_mask[:, :, :])
            i16 = cp.tile([128, 128], F16)
            nc.sync.dma_start(i16[:], id16_in[:, :])
            big16 = cp.tile([128, 1733], F16)
            nc.sync.dma_start(big16[:], big16_in[:, :])
            big32 = cp.tile([128, 106], F32)
            nc.sync.dma_start(big32[:], big32_in[:, :])
            wgt = cp.tile([1, NJ], F32)
            nc.sync.dma_start(wgt[:], wgt_in[:, :])
            f2t = []
            for m in range(3):
                t = cp.tile([100, NCLS], F16, tag=f"f2_{m}", name=f"f2sb{m}")
                nc.sync.dma_start(t[:], f2_pack[100 * m:100 * (m + 1), :])
                f2t.append(t)
            t = cp.tile([101, NCLS], F16, tag="f2_3", name="f2sb3")
            nc.sync.dma_start(t[:], f2_pack[300:401, :])
            f2t.append(t)

            cbw = big16[0:D, 0:33]
            cw = big16[0:D, 33:633]
            mfw = big16[0:D, 633:933]
            fk0 = big16[0:100, 933:1333]
            fk1 = big16[0:101, 1333:1733]
            a2w = big32[0:D, 0:100]
            a2b = big32[0:LOCAL, 100:101]
            mfb = big32[0:GOUT, 101:102]
            cb = big32[0:NF, 102:105]
            abt = big32[0:WIN, 105:106]

            ones5 = cp.tile([WIN, 1], F16)
            nc.vector.memset(ones5[:], 1.0)
            ones_k1 = cp.tile([1, D], F32)
            nc.vector.memset(ones_k1[:], 1.0)

            # parity-select then transpose chunk c into EG[c] = [D, n]
            EG = [cp.tile([128, NCH[c]], F16, tag=f"EG{c}", name=f"EG{c}")
                  for c in range(len(NCH))]

            def fill_eg(c):
                ntile = NCH[c] // 128
                t0 = IOFF[c] // 128
                nc.vector.copy_predicated(
                    G[c][:, :, 0:D], gmk[:, t0:t0 + ntile, 0:D],
                    G[c][:, :, 128:128 + D])
                for j0 in range(0, ntile, 8):
                    jn = min(8, ntile - j0)
                    pe = psB.tile([D, 1024], F16, tag="tp")
                    for j in range(jn):
                        nc.tensor.transpose(pe[:, j * 128:(j + 1) * 128],
                                            G[c][:, j0 + j, 0:D], i16[:])
                    nc.scalar.copy(EG[c][0:D, j0 * 128:(j0 + jn) * 128],
                                   pe[:, 0:jn * 128])

            # ---- tail chunk: scores (true tanh), Asum windows, AllGather ----
            # combo col 32 = 1/CR so judge compares ss' > (max+min) directly
            fill_eg(0)
            ET = EG[0]                            # [128(D), 256] f16
            xs = cp.tile([WIN, NTAIL], F16)
            ss = cp.tile([1, NTAIL], F32)
            ps = psM.tile([128, 512], F32, tag="m")
            nc.tensor.matmul(ps[0:33, 0:NTAIL], cbw, ET[0:D, :])
            nc.scalar.activation(xs[:], ps[0:WIN, 0:NTAIL], AF.Tanh,
                                 bias=abt)
            nc.scalar.copy(ss[:], ps[32:33, 0:NTAIL])
            asum = cp.tile([1, NTAIL], F32)
            asneg = cp.tile([1, NTAIL], F32)
            ps = psM.tile([128, 512], F32, tag="m")
            nc.tensor.matmul(ps[0:1, 0:NTAIL], ones5[:], xs[:])
            nc.scalar.copy(asum[:], ps[0:1, 0:NTAIL])
            nc.scalar.activation(asneg[:], ps[0:1, 0:NTAIL], AF.Identity,
                                 scale=-1.0)
            partial = cp.tile([1, 2 * TBLK], F32)
            nc.vector.reduce_max(
                partial[0:1, 0:TBLK],
                asum[0:1, :].rearrange("p (a b) -> p a b", b=BSH), axis=AX.X)
            nc.vector.reduce_max(
                partial[0:1, TBLK:2 * TBLK],
                asneg[0:1, :].rearrange("p (a b) -> p a b", b=BSH), axis=AX.X)
            cc_in = dp.tile([1, 2 * TBLK], F32)
            cc_out = dp.tile([NCORE, 2 * TBLK], F32)
            nc.sync.dma_start(cc_in[:], partial[:])
            nc.gpsimd.collective_compute(
                "AllGather", ALU.bypass,
                replica_groups=[list(range(NCORE))],
                ins=[cc_in.opt()], outs=[cc_out.opt()])
            gm = cp.tile([1, NCORE, 2 * TBLK], F32)
            nc.sync.dma_start(gm[:], cc_out[:, :].unsqueeze(0))

            # ---- remaining gathers (queue behind the collective trigger) ----
            for c in range(2, len(NCH)):
                gather(c)

            # ---- CNN chunks ----
            pool = [cp.tile([NF, BSH], F32, tag=f"pool{k}", name=f"pool{k}")
                    for k in range(3)]
            taps = [(0, 1), (1, 2), (3, 3)]
            for c in range(1, len(NCH)):
                fill_eg(c)
                # pair-tap-major: each conv weight is loaded once per row
                # pair instead of once per row (halves LDWEIGHTS traffic)
                for lr0 in range(0, ROWS[c - 1], 2):
                    prs = [lr for lr in (lr0, lr0 + 1) if lr < ROWS[c - 1]]
                    pcs = {}
                    for k, (t0, ntap) in enumerate(taps):
                        T = LG - ntap + 1
                        for lr in prs:
                            pcs[(k, lr)] = psC.tile(
                                [NF, 416], F32, tag=f"c{k}p{lr % 2}",
                                name=f"pc{c}_{k}_{lr}")
                        for j in range(ntap):
                            for lr in prs:
                                base = lr * LG
                                nc.tensor.matmul(
                                    pcs[(k, lr)][:, 0:T],
                                    cw[:, (t0 + j) * 100:(t0 + j + 1) * 100],
                                    EG[c][0:D, base + j:base + j + T],
                                    start=(j == 0), stop=(j == ntap - 1))
                    for k, (t0, ntap) in enumerate(taps):
                        T = LG - ntap + 1
                        for lr in prs:
                            r = ROFF[c - 1] + lr
                            nc.vector.reduce_max(pool[k][:, r:r + 1],
                                                 pcs[(k, lr)][:, 0:T],
                                                 axis=AX.X)

            # ---- judge + local units (AllGather long since done) ----
            gmax = cp.tile([1, 2 * TBLK], F32)
            nc.vector.reduce_max(gmax[:], gm[:].rearrange("p g t -> p t g"),
                                 axis=AX.X)
            wmax = cp.tile([1, NP_TAIL], F32)
            wneg = cp.tile([1, NP_TAIL], F32)
            nc.vector.reduce_max(wmax[:], _win_ap(gmax, 0), axis=AX.X)
            nc.vector.reduce_max(wneg[:], _win_ap(gmax, TBLK), axis=AX.X)
            cmp = cp.tile([1, NP_TAIL], F32)
            nc.vector.tensor_sub(cmp[:], wmax[:], wneg[:])
            judge = cp.tile([1, NJ], F32)
            nc.vector.tensor_tensor(
                judge[0:1, :].rearrange("p (a b) -> p a b", b=BSH),
                ss[0:1, JOFF:JOFF + NJ].rearrange("p (a b) -> p a b", b=BSH),
                cmp[0:1, :].unsqueeze(2).broadcast_to([1, NP_TAIL, BSH]),
                op=ALU.is_gt)
            nc.vector.tensor_mul(judge[:], judge[:], wgt[:])
            ET32 = cp.tile([D, NJ], F32)
            nc.scalar.copy(ET32[:], ET[0:D, JOFF:JOFF + NJ])
            jb = psM.tile([128, 512], F32, tag="m")
            nc.tensor.matmul(jb[0:D, 0:NJ], ones_k1[:], judge[:])
            sET = cp.tile([D, NJ], F32)
            nc.vector.tensor_tensor(sET[:], ET32[:], jb[0:D, 0:NJ], op=ALU.mult)
            twT = cp.tile([D, BSH], F32)
            nc.vector.reduce_sum(
                twT[:], sET[:].rearrange("p (blk b) -> p b blk", b=BSH),
                axis=AX.X)
            lup = psM.tile([128, 512], F32, tag="m")
            nc.tensor.matmul(lup[0:LOCAL, 0:BSH], a2w, twT[:])
            luT = cp.tile([LOCAL, BSH], F16)
            nc.scalar.activation(luT[:], lup[0:LOCAL, 0:BSH], AF.Identity,
                                 bias=a2b)

            # ---- head ----
            poolr = [cp.tile([NF, BSH], F16, tag=f"poolr{k}", name=f"poolr{k}")
                     for k in range(3)]
            for k in range(3):
                nc.scalar.activation(poolr[k][:], pool[k][:], AF.Relu,
                                     bias=cb[:, k:k + 1])
            gup = psM.tile([128, 512], F32, tag="m")
            for k in range(3):
                nc.tensor.matmul(gup[0:GOUT, 0:BSH], mfw[:, 100 * k:100 * (k + 1)],
                                 poolr[k][:], start=(k == 0), stop=(k == 2))
            guT = cp.tile([GOUT + 1, BSH], F16)
            nc.vector.memset(guT[:], 1.0)
            nc.scalar.activation(guT[0:GOUT, :], gup[0:GOUT, 0:BSH], AF.Identity,
                                 bias=mfb)
            hT = [cp.tile([100 + (m == 3), BSH], F16, tag=f"h{m}", name=f"hT{m}")
                  for m in range(4)]
            nc.vector.memset(hT[3][:], 1.0)
            for m in range(4):
                hp = psM.tile([128, 512], F32, tag="m")
                nc.tensor.matmul(hp[0:100, 0:BSH], fk0[:, 100 * m:100 * (m + 1)],
                                 luT[:], start=True, stop=False)
                nc.tensor.matmul(hp[0:100, 0:BSH], fk1[:, 100 * m:100 * (m + 1)],
                                 guT[:], start=False, stop=True)
                nc.scalar.activation(hT[m][0:100, :], hp[0:100, 0:BSH], AF.Relu)
            out_sb = cp.tile([BSH, NCLS], F32)
            for n0, nn in N_TILES_OUT:
                op_ = psM.tile([128, 512], F32, tag="m")
                for m in range(4):
                    nc.tensor.matmul(op_[0:BSH, 0:nn], hT[m][:],
                                     f2t[m][:, n0:n0 + nn],
                                     start=(m == 0), stop=(m == 3))
                nc.scalar.copy(out_sb[:, n0:n0 + nn], op_[0:BSH, 0:nn])
            nc.sync.dma_start(y[:, :], out_sb[:])

    nc.compile()
    return nc


def _prep(inputs):
    """host-side packing; returns per-core in_maps."""
    emb = np.asarray(inputs["emb"], np.float32)
    l_txt = np.asarray(inputs["l_train_text"])
    g_txt = np.asarray(inputs["g_train_text"])

    pairs16 = np.zeros((VPAIR, EPAIR), np.float16)
    pairs16[:, 0:D] = emb[0::2]
    pairs16[:, 128:128 + D] = emb[1::2]

    big16 = np.zeros((128, 1733), np.float16)
    att_w = np.asarray(inputs["att_w"], np.float32)
    big16[0:D, 0:WIN] = att_w.T
    big16[0:D, 32] = 1.0 / CR          # ss' = centers_sum / CR
    big16[0:D, 33:133] = np.asarray(inputs["conv1_w"])[:, 0, 0, :].T
    big16[0:D, 133:233] = np.asarray(inputs["conv2_w"])[:, 0, 0, :].T
    big16[0:D, 233:333] = np.asarray(inputs["conv2_w"])[:, 0, 1, :].T
    big16[0:D, 333:433] = np.asarray(inputs["conv3_w"])[:, 0, 0, :].T
    big16[0:D, 433:533] = np.asarray(inputs["conv3_w"])[:, 0, 1, :].T
    big16[0:D, 533:633] = np.asarray(inputs["conv3_w"])[:, 0, 2, :].T
    mf_w = np.asarray(inputs["mf_w"], np.float32)
    for c in range(3):
        big16[0:D, 633 + 100 * c:733 + 100 * c] = mf_w[:, 100 * c:100 * (c + 1)].T
    fin_w = np.asarray(inputs["fin_w"], np.float32)
    big16[0:100, 933:1333] = fin_w.T[0:100]
    big16[0:100, 1333:1733] = fin_w.T[100:200]
    big16[100, 1333:1733] = np.asarray(inputs["fin_b"], np.float32)

    big32 = np.zeros((128, 106), np.float32)
    big32[0:D, 0:100] = np.asarray(inputs["att2_w"], np.float32).T
    big32[0:LOCAL, 100] = np.asarray(inputs["att2_b"], np.float32)
    big32[0:GOUT, 101] = np.asarray(inputs["mf_b"], np.float32)
    big32[0:NF, 102] = np.asarray(inputs["conv1_b"], np.float32)
    big32[0:NF, 103] = np.asarray(inputs["conv2_b"], np.float32)
    big32[0:NF, 104] = np.asarray(inputs["conv3_b"], np.float32)
    big32[0:WIN, 105] = np.asarray(inputs["att_b"], np.float32)

    f2p = np.zeros((401, NCLS), np.float16)
    f2p[0:400] = np.asarray(inputs["fin2_w"], np.float32).T
    f2p[400] = np.asarray(inputs["fin2_b"], np.float32)

    # tw weights: w_p = P^-(P-p), p = P0 + col//BSH
    wgt = np.zeros((1, NJ), np.float32)
    for k in range(NP_TAIL):
        wgt[0, k * BSH:(k + 1) * BSH] = np.float64(P) ** -(NP_TAIL - k)

    shared = {
        "pairs16": pairs16,
        "id16": np.eye(128, dtype=np.float16),
        "wgt": wgt,
        "big16": big16,
        "big32": big32,
        "f2_pack": f2p,
    }

    in_maps = []
    for core in range(NCORE):
        ls = l_txt[core * BSH:(core + 1) * BSH]
        gs = g_txt[core * BSH:(core + 1) * BSH]
        # tail slots: col = blk*BSH + b, t = P0 + blk
        blk = np.arange(NTAIL) // BSH
        bb = np.arange(NTAIL) % BSH
        toks = [ls[bb, P0 + blk].astype(np.int64)]
        for ci, nr in enumerate(ROWS):
            t = gs[ROFF[ci]:ROFF[ci] + nr].reshape(-1).astype(np.int64)
            toks.append(np.concatenate(
                [t, np.zeros(NCH[ci + 1] - nr * LG, np.int64)]))
        all_toks = np.concatenate(toks)
        idx_cols = [_wrap_idx(t >> 1) for t in toks]
        g_idx_arr = np.concatenate(idx_cols, axis=1)
        par = (all_toks & 1).astype(np.uint8)
        g_mask_arr = np.broadcast_to(
            par.reshape(NIDX // 128, 128).T[:, :, None],
            (128, NIDX // 128, 128)).copy()
        m = dict(shared)
        m["g_idx"] = g_idx_arr
        m["g_mask"] = g_mask_arr
        in_maps.append(m)
    return in_maps


def _run(inputs, trace=False, tmpdir=None):
    if "nc" not in _CACHE:
        _CACHE["nc"] = _build()
    nc = _CACHE["nc"]
    in_maps = _prep(inputs)
    res = run_bass_kernel_spmd(nc, in_maps, list(range(NCORE)),
                               trace=trace, tmpdir=tmpdir)
    out = np.concatenate([res.results[i]["y"] for i in range(NCORE)], axis=0)
    return out, res


def kernel(**inputs):
    out, _ = _run(inputs, trace=False)
    return out
